# revision 1
# baseline (speedup 1.0000x reference)
"""Fused multi-head attention (QKV projection + softmax attention) on 8 TRN2
NeuronCores.

Problem: x [2, 2048, 1024] f32, w_qkv [1024, 3072] f32 ->
         out [2, 16, 2048, 64] f32   (16 heads, head_dim 64)

Sharding: tensor-parallel over heads. Each of the 8 cores owns 2 heads: it
gets the full x plus its [1024, 384] slice of w_qkv columns (q|k|v for its 2
heads) and computes its slice of the output independently. No collectives.

Per-core pipeline (all matmuls in float32r = full-rate fp32, ~1e-4 rel err):
  1. PE-transpose x[b] -> xT [d, n]  (PE matmul contracts over the partition
     dim, so both x@w operands must be d-major).
  2. QKV^T [e, n] = w.T @ x.T via PE, e-blocks of 128 (= 2 heads * 64).
  3. Attention per (b, head, q-half) with *transposed* scores:
       S^T[k, q] = K^T.T @ Q^T    (softmax needs no max-subtraction: scores
       exp(S^T/8) -> P^T           are ~N(0,1), far from fp32 overflow)
       O^T[d, q], denom[q] = [V | 1]^T.T @ P^T   (ones column makes the
                                                  softmax denominators a free
                                                  65th output row)
     PE-transpose O^T -> O [q, 65], then per-row multiply by 1/denom.
"""

import numpy as np

import concourse.bass as bass
import concourse.tile as tile
from concourse import bacc, mybir
from concourse.bass_utils import run_bass_kernel_spmd
from concourse.masks import make_identity

F32 = mybir.dt.float32
F32R = mybir.dt.float32r

B = 2
N = 2048
DIM = 1024
HEADS = 16
HD = 64
NCORES = 8
H_LOC = HEADS // NCORES  # 2 heads per core
E = 3 * H_LOC * HD       # 384 w columns per core


def _emit(tc, out_ap, x_ap, w_ap):
    nc = tc.nc
    from contextlib import ExitStack
    ctx = ExitStack()
    with ctx:
        const = ctx.enter_context(tc.tile_pool(name="const", bufs=1))
        xtp = ctx.enter_context(tc.tile_pool(name="xtp", bufs=1))
        xst = ctx.enter_context(tc.tile_pool(name="xst", bufs=3))
        qkv = ctx.enter_context(tc.tile_pool(name="qkv", bufs=1))
        ptp = ctx.enter_context(tc.tile_pool(name="ptp", bufs=3))
        osb = ctx.enter_context(tc.tile_pool(name="osb", bufs=2))
        onp = ctx.enter_context(tc.tile_pool(name="onp", bufs=2))
        smp = ctx.enter_context(tc.tile_pool(name="smp", bufs=4))
        psS = ctx.enter_context(tc.tile_pool(name="psS", bufs=2, space="PSUM"))
        psO = ctx.enter_context(tc.tile_pool(name="psO", bufs=2, space="PSUM"))

        ident = const.tile([128, 128], F32)
        make_identity(nc, ident)
        ones = const.tile([128, 1], F32)
        nc.vector.memset(ones, 1.0)

        # w [1024, 384] -> [128, 8, 384], rounded to f32r for the PE
        w_sb = const.tile([128, 8, E], F32)
        nc.sync.dma_start(out=w_sb, in_=w_ap.rearrange("(c p) e -> p c e", p=128))
        w_r = const.tile([128, 8, E], F32R)
        nc.vector.tensor_copy(out=w_r, in_=w_sb)

        for b in range(B):
            # ---- transpose x[b] [2048, 1024] -> xT [128, 8, 2048] (d-major)
            xT = xtp.tile([128, 8, N], F32R, tag="xT")
            for nb in range(16):
                xs = xst.tile([128, DIM], F32, tag="xs")
                nc.sync.dma_start(out=xs, in_=x_ap[b, nb * 128:(nb + 1) * 128, :])
                for dc in range(8):
                    tp = psS.tile([128, 128], F32, tag="S")
                    nc.tensor.transpose(tp, xs[:, dc * 128:(dc + 1) * 128], ident)
                    nc.vector.tensor_copy(
                        out=xT[:, dc, nb * 128:(nb + 1) * 128], in_=tp)

            # ---- QKV^T [e, n]: for each 128-wide e-block (q | k | v)
            qT = qkv.tile([128, N], F32R, tag="qT")
            kT = qkv.tile([128, N], F32R, tag="kT")
            vT = qkv.tile([128, N], F32, tag="vT")
            for eb in range(3):
                tgt = (qT, kT, vT)[eb]
                for nc4 in range(4):
                    ps = psO.tile([128, 512], F32, tag="O")
                    for dc in range(8):
                        nc.tensor.matmul(
                            ps,
                            w_r[:, dc, eb * 128:(eb + 1) * 128],
                            xT[:, dc, nc4 * 512:(nc4 + 1) * 512],
                            start=(dc == 0), stop=(dc == 7))
                    nc.vector.tensor_copy(out=tgt[:, nc4 * 512:(nc4 + 1) * 512],
                                          in_=ps)

            # ---- V' = [V_h | ones] per head, natural [k, d] layout
            Vp = qkv.tile([128, 16, 130], F32R, tag="Vp")
            for kb in range(16):
                tp = psS.tile([128, 128], F32, tag="S")
                nc.tensor.transpose(tp, vT[:, kb * 128:(kb + 1) * 128], ident)
                nc.vector.tensor_copy(out=Vp[:, kb, 0:64], in_=tp[:, 0:64])
                nc.vector.tensor_copy(out=Vp[:, kb, 65:129], in_=tp[:, 64:128])
                nc.vector.tensor_copy(out=Vp[:, kb, 64:65], in_=ones)
                nc.vector.tensor_copy(out=Vp[:, kb, 129:130], in_=ones)

            # ---- attention per (head, q-half of 1024)
            for h in range(H_LOC):
                hp = 64 * h
                for qh in range(2):
                    OT = psO.tile([65, 1024], F32, tag="O")
                    for kb in range(16):
                        S = psS.tile([128, 1024], F32, tag="S")
                        for qc in range(2):
                            q0 = qh * 1024 + qc * 512
                            nc.tensor.matmul(
                                S[:, qc * 512:(qc + 1) * 512],
                                kT[hp:hp + 64, kb * 128:(kb + 1) * 128],
                                qT[hp:hp + 64, q0:q0 + 512],
                                start=True, stop=True)
                        PT = ptp.tile([128, 1024], F32R, tag="PT")
                        nc.scalar.activation(
                            out=PT, in_=S,
                            func=mybir.ActivationFunctionType.Exp, scale=0.125)
                        for qc in range(2):
                            nc.tensor.matmul(
                                OT[:, qc * 512:(qc + 1) * 512],
                                Vp[:, kb, 65 * h:65 * h + 65],
                                PT[:, qc * 512:(qc + 1) * 512],
                                start=(kb == 0), stop=(kb == 15))
                    # epilogue: transpose back to [q, 65], normalize, store
                    OT_sb = osb.tile([65, 1024], F32, tag="OT")
                    nc.vector.tensor_copy(out=OT_sb, in_=OT)
                    On = onp.tile([128, 8, HD], F32, tag="On")
                    for t in range(8):
                        tp = psS.tile([128, 65], F32, tag="S")
                        nc.tensor.transpose(
                            tp, OT_sb[:, t * 128:(t + 1) * 128],
                            ident[0:65, 0:65])
                        rec = smp.tile([128, 1], F32, tag="rec")
                        nc.vector.reciprocal(rec, tp[:, 64:65])
                        nc.vector.tensor_scalar_mul(On[:, t, :], tp[:, 0:64], rec)
                    nc.sync.dma_start(
                        out=out_ap[b, h, qh * 1024:(qh + 1) * 1024, :]
                        .rearrange("(t p) d -> p t d", p=128),
                        in_=On)


_CACHED_NC = None


def _build():
    global _CACHED_NC
    if _CACHED_NC is not None:
        return _CACHED_NC
    nc = bacc.Bacc("TRN2", target_bir_lowering=False, debug=False,
                   num_devices=NCORES)
    x = nc.dram_tensor("x", [B, N, DIM], F32, kind="ExternalInput").ap()
    w = nc.dram_tensor("w", [DIM, E], F32, kind="ExternalInput").ap()
    out = nc.dram_tensor("out", [B, H_LOC, N, HD], F32,
                         kind="ExternalOutput").ap()
    with tile.TileContext(nc) as tc:
        _emit(tc, out, x, w)
    nc.compile()
    _CACHED_NC = nc
    return nc


def _w_slice(w_qkv: np.ndarray, core: int) -> np.ndarray:
    cols = []
    for part in range(3):  # q, k, v column groups of w_qkv
        base = part * HEADS * HD + core * H_LOC * HD
        cols.append(w_qkv[:, base:base + H_LOC * HD])
    return np.ascontiguousarray(np.concatenate(cols, axis=1))


def kernel(x: np.ndarray, w_qkv: np.ndarray, _trace: bool = False):
    nc = _build()
    x = np.ascontiguousarray(x, dtype=np.float32)
    in_maps = [{"x": x, "w": _w_slice(w_qkv, i)} for i in range(NCORES)]
    res = run_bass_kernel_spmd(nc, in_maps, list(range(NCORES)), trace=_trace)
    out = np.empty((B, HEADS, N, HD), np.float32)
    for i in range(NCORES):
        out[:, i * H_LOC:(i + 1) * H_LOC] = res.results[i]["out"]
    if _trace:
        kernel.last_exec_time_ns = res.exec_time_ns
    return out


# revision 5
# speedup vs baseline: 1.0235x; 1.0235x over previous
"""Fused multi-head attention (QKV projection + softmax attention) on 8 TRN2
NeuronCores.

Problem: x [2, 2048, 1024] f32, w_qkv [1024, 3072] f32 ->
         out [2, 16, 2048, 64] f32   (16 heads, head_dim 64)

Sharding: tensor-parallel over heads. Each of the 8 cores owns 2 heads: it
gets the full x plus its [1024, 384] slice of w_qkv columns (q|k|v for its 2
heads) and computes its slice of the output independently. No collectives.

Per-core pipeline (all matmuls in float32r = full-rate fp32, ~1e-4 rel err):
  1. PE-transpose x[b] -> xT [d, n]  (PE matmul contracts over the partition
     dim, so both x@w operands must be d-major).
  2. QKV^T [e, n] = w.T @ x.T via PE, e-blocks of 128 (= 2 heads * 64).
  3. Attention per (b, head, q-half) with *transposed* scores:
       S^T[k, q] = K^T.T @ Q^T    (softmax needs no max-subtraction: scores
       exp(S^T/8) -> P^T           are ~N(0,1), far from fp32 overflow)
       O^T[d, q], denom[q] = [V | 1]^T.T @ P^T   (ones column makes the
                                                  softmax denominators a free
                                                  65th output row)
     PE-transpose O^T -> O [q, 65], then per-row multiply by 1/denom.
"""

import numpy as np

import concourse.bass as bass
import concourse.tile as tile
from concourse import bacc, mybir
from concourse.bass_utils import run_bass_kernel_spmd
from concourse.masks import make_identity

F32 = mybir.dt.float32
F32R = mybir.dt.float32r
BF16 = mybir.dt.bfloat16
PV_DT = BF16  # dtype of P^T and V' for the P@V matmul

B = 2
N = 2048
DIM = 1024
HEADS = 16
HD = 64
NCORES = 8
H_LOC = HEADS // NCORES  # 2 heads per core
E = 3 * H_LOC * HD       # 384 w columns per core


def _emit(tc, out_ap, x_ap, w_ap):
    nc = tc.nc
    from contextlib import ExitStack
    ctx = ExitStack()
    with ctx:
        const = ctx.enter_context(tc.tile_pool(name="const", bufs=1))
        xtp = ctx.enter_context(tc.tile_pool(name="xtp", bufs=1))
        xst = ctx.enter_context(tc.tile_pool(name="xst", bufs=3))
        qkv = ctx.enter_context(tc.tile_pool(name="qkv", bufs=1))
        ptp = ctx.enter_context(tc.tile_pool(name="ptp", bufs=4))
        osb = ctx.enter_context(tc.tile_pool(name="osb", bufs=2))
        onp = ctx.enter_context(tc.tile_pool(name="onp", bufs=2))
        smp = ctx.enter_context(tc.tile_pool(name="smp", bufs=4))
        psS = ctx.enter_context(tc.tile_pool(name="psS", bufs=2, space="PSUM"))
        psO = ctx.enter_context(tc.tile_pool(name="psO", bufs=2, space="PSUM"))

        ident = const.tile([128, 128], F32)
        make_identity(nc, ident)
        ones = const.tile([128, 1], F32)
        nc.vector.memset(ones, 1.0)

        # w [1024, 384] -> [128, 8, 384], rounded to f32r for the PE
        w_sb = const.tile([128, 8, E], F32)
        nc.sync.dma_start(out=w_sb, in_=w_ap.rearrange("(c p) e -> p c e", p=128))
        w_r = const.tile([128, 8, E], F32R)
        nc.vector.tensor_copy(out=w_r, in_=w_sb)

        for b in range(B):
            # ---- transpose x[b] [2048, 1024] -> xT [128, 8, 2048] (d-major)
            xT = xtp.tile([128, 8, N], F32R, tag="xT")
            for nb in range(16):
                xs = xst.tile([128, DIM], F32, tag="xs")
                nc.sync.dma_start(out=xs, in_=x_ap[b, nb * 128:(nb + 1) * 128, :])
                for dc in range(8):
                    tp = psS.tile([128, 128], F32, tag="S")
                    nc.tensor.transpose(tp, xs[:, dc * 128:(dc + 1) * 128], ident)
                    nc.vector.tensor_copy(
                        out=xT[:, dc, nb * 128:(nb + 1) * 128], in_=tp)

            # ---- QKV^T [e, n]: for each 128-wide e-block (q | k | v)
            qT = qkv.tile([128, N], F32R, tag="qT")
            kT = qkv.tile([128, N], F32R, tag="kT")
            vT = qkv.tile([128, N], F32, tag="vT")
            for eb in range(3):
                tgt = (qT, kT, vT)[eb]
                for nc4 in range(4):
                    ps = psO.tile([128, 512], F32, tag="O")
                    for dc in range(8):
                        nc.tensor.matmul(
                            ps,
                            w_r[:, dc, eb * 128:(eb + 1) * 128],
                            xT[:, dc, nc4 * 512:(nc4 + 1) * 512],
                            start=(dc == 0), stop=(dc == 7))
                    nc.vector.tensor_copy(out=tgt[:, nc4 * 512:(nc4 + 1) * 512],
                                          in_=ps)

            # ---- V' = [V_h | ones] per head, natural [k, d] layout
            Vp = qkv.tile([128, 16, 130], PV_DT, tag="Vp")
            for kb in range(16):
                tp = psS.tile([128, 128], F32, tag="S")
                nc.tensor.transpose(tp, vT[:, kb * 128:(kb + 1) * 128], ident)
                nc.vector.tensor_copy(out=Vp[:, kb, 0:64], in_=tp[:, 0:64])
                nc.vector.tensor_copy(out=Vp[:, kb, 65:129], in_=tp[:, 64:128])
                nc.vector.tensor_copy(out=Vp[:, kb, 64:65], in_=ones)
                nc.vector.tensor_copy(out=Vp[:, kb, 129:130], in_=ones)

            # ---- attention: both heads interleaved (disjoint PE row groups
            # -> the two heads' K=64 score matmuls run concurrently)
            for qh in range(2):
                OT = [psO.tile([65, 1024], F32, tag="O", name=f"OT{_h}")
                      for _h in range(H_LOC)]
                for kb in range(16):
                    S = [psS.tile([128, 1024], F32, tag="S", name=f"S{_h}")
                         for _h in range(H_LOC)]
                    for qc in range(2):
                        q0 = qh * 1024 + qc * 512
                        for h in range(H_LOC):
                            hp = 64 * h
                            nc.tensor.matmul(
                                S[h][:, qc * 512:(qc + 1) * 512],
                                kT[hp:hp + 64, kb * 128:(kb + 1) * 128],
                                qT[hp:hp + 64, q0:q0 + 512],
                                start=True, stop=True)
                    PT = [ptp.tile([128, 1024], PV_DT, tag="PT", name=f"PT{_h}")
                          for _h in range(H_LOC)]
                    for h in range(H_LOC):
                        nc.scalar.activation(
                            out=PT[h], in_=S[h],
                            func=mybir.ActivationFunctionType.Exp, scale=0.125)
                    for qc in range(2):
                        for h in range(H_LOC):
                            nc.tensor.matmul(
                                OT[h][:, qc * 512:(qc + 1) * 512],
                                Vp[:, kb, 65 * h:65 * h + 65],
                                PT[h][:, qc * 512:(qc + 1) * 512],
                                start=(kb == 0), stop=(kb == 15))
                # epilogue: transpose back to [q, 65], normalize, store
                for h in range(H_LOC):
                    OT_sb = osb.tile([65, 1024], F32, tag="OT")
                    nc.vector.tensor_copy(out=OT_sb, in_=OT[h])
                    On = onp.tile([128, 8, HD], F32, tag="On")
                    for t in range(8):
                        tp = psS.tile([128, 65], F32, tag="S")
                        nc.tensor.transpose(
                            tp, OT_sb[:, t * 128:(t + 1) * 128],
                            ident[0:65, 0:65])
                        rec = smp.tile([128, 1], F32, tag="rec")
                        nc.vector.reciprocal(rec, tp[:, 64:65])
                        nc.vector.tensor_scalar_mul(On[:, t, :], tp[:, 0:64], rec)
                    nc.sync.dma_start(
                        out=out_ap[b, h, qh * 1024:(qh + 1) * 1024, :]
                        .rearrange("(t p) d -> p t d", p=128),
                        in_=On)


_CACHED_NC = None


def _build():
    global _CACHED_NC
    if _CACHED_NC is not None:
        return _CACHED_NC
    nc = bacc.Bacc("TRN2", target_bir_lowering=False, debug=False,
                   num_devices=NCORES)
    x = nc.dram_tensor("x", [B, N, DIM], F32, kind="ExternalInput").ap()
    w = nc.dram_tensor("w", [DIM, E], F32, kind="ExternalInput").ap()
    out = nc.dram_tensor("out", [B, H_LOC, N, HD], F32,
                         kind="ExternalOutput").ap()
    with tile.TileContext(nc) as tc:
        _emit(tc, out, x, w)
    nc.compile()
    _CACHED_NC = nc
    return nc


def _w_slice(w_qkv: np.ndarray, core: int) -> np.ndarray:
    cols = []
    for part in range(3):  # q, k, v column groups of w_qkv
        base = part * HEADS * HD + core * H_LOC * HD
        cols.append(w_qkv[:, base:base + H_LOC * HD])
    return np.ascontiguousarray(np.concatenate(cols, axis=1))


def kernel(x: np.ndarray, w_qkv: np.ndarray, _trace: bool = False):
    nc = _build()
    x = np.ascontiguousarray(x, dtype=np.float32)
    in_maps = [{"x": x, "w": _w_slice(w_qkv, i)} for i in range(NCORES)]
    res = run_bass_kernel_spmd(nc, in_maps, list(range(NCORES)), trace=_trace)
    out = np.empty((B, HEADS, N, HD), np.float32)
    for i in range(NCORES):
        out[:, i * H_LOC:(i + 1) * H_LOC] = res.results[i]["out"]
    if _trace:
        kernel.last_exec_time_ns = res.exec_time_ns
    return out


# revision 10
# speedup vs baseline: 1.0972x; 1.0720x over previous
"""Fused multi-head attention (QKV projection + softmax attention) on 8 TRN2
NeuronCores.

Problem: x [2, 2048, 1024] f32, w_qkv [1024, 3072] f32 ->
         out [2, 16, 2048, 64] f32   (16 heads, head_dim 64)

Sharding: tensor-parallel over heads. Each of the 8 cores owns 2 heads: it
gets the full x plus its [1024, 384] slice of w_qkv columns (q|k|v for its 2
heads) and computes its slice of the output independently. No collectives.

Per-core pipeline (all matmuls in float32r = full-rate fp32, ~1e-4 rel err):
  1. PE-transpose x[b] -> xT [d, n]  (PE matmul contracts over the partition
     dim, so both x@w operands must be d-major).
  2. QKV^T [e, n] = w.T @ x.T via PE, e-blocks of 128 (= 2 heads * 64).
  3. Attention with *transposed* scores, both heads interleaved so their
     K=64 score matmuls occupy disjoint PE row groups (concurrent) and the
     M=64 value matmuls occupy disjoint PE column groups (concurrent):
       S^T[k, q] = K^T.T @ Q^T     (no max-subtraction needed: scores are
       exp(S^T/8) -> P^T            ~N(0,1), far from fp32 overflow)
       O^T[d, q] (+= over k-blocks) = V.T @ P^T   both heads in one psum tile
       denom[q]  (+= over k-blocks) = ones.T @ P^T  [33,1024] psum, rows 0/32
     PE-transpose O^T and denom back to [q, *], multiply by 1/denom per row.
"""

import numpy as np

import concourse.bass as bass
import concourse.tile as tile
from concourse import bacc, mybir
from concourse.bass_utils import run_bass_kernel_spmd
from concourse.masks import make_identity

F32 = mybir.dt.float32
F32R = mybir.dt.float32r

B = 2
N = 2048
DIM = 1024
HEADS = 16
HD = 64
NCORES = 8
H_LOC = HEADS // NCORES  # 2 heads per core
E = 3 * H_LOC * HD       # 384 w columns per core
CACHEBUST = 4


def _emit(tc, out_ap, x_ap, w_ap):
    nc = tc.nc
    from contextlib import ExitStack
    ctx = ExitStack()
    with ctx:
        const = ctx.enter_context(tc.tile_pool(name="const", bufs=1))
        xtp = ctx.enter_context(tc.tile_pool(name="xtp", bufs=1))
        xst = ctx.enter_context(tc.tile_pool(name="xst", bufs=3))
        qkv = ctx.enter_context(tc.tile_pool(name="qkv", bufs=1))
        ptp = ctx.enter_context(tc.tile_pool(name="ptp", bufs=4))
        osb = ctx.enter_context(tc.tile_pool(name="osb", bufs=3))
        onp = ctx.enter_context(tc.tile_pool(name="onp", bufs=2))
        smp = ctx.enter_context(tc.tile_pool(name="smp", bufs=4))

        ident = const.tile([128, 128], F32)
        make_identity(nc, ident)
        ones = const.tile([128, 1], F32)
        nc.vector.memset(ones, 1.0)
        ones_r = const.tile([128, 1], F32R)
        nc.vector.tensor_copy(out=ones_r, in_=ones)
        _cb = const.tile([128, 1], F32, name="cb")
        nc.vector.memset(_cb, float(CACHEBUST))

        # w [1024, 384] -> [128, 8, 384], rounded to f32r for the PE
        w_sb = const.tile([128, 8, E], F32)
        nc.sync.dma_start(out=w_sb, in_=w_ap.rearrange("(c p) e -> p c e", p=128))
        w_r = const.tile([128, 8, E], F32R)
        nc.vector.tensor_copy(out=w_r, in_=w_sb)

        for b in range(B):
            # ============ projection phase (scoped PSUM pools) ============
            with tc.tile_pool(name=f"psT{b}", bufs=3, space="PSUM") as psT, \
                 tc.tile_pool(name=f"psQ{b}", bufs=2, space="PSUM") as psQ:
                # -- transpose x[b] [2048, 1024] -> xT [128, 8, 2048]
                xT = xtp.tile([128, 8, N], F32R, tag="xT")
                for nb in range(16):
                    xs = xst.tile([128, DIM], F32, tag="xs")
                    nc.sync.dma_start(out=xs,
                                      in_=x_ap[b, nb * 128:(nb + 1) * 128, :])
                    for dc in range(8):
                        tp = psT.tile([128, 128], F32, tag="tp")
                        nc.tensor.transpose(tp, xs[:, dc * 128:(dc + 1) * 128],
                                            ident)
                        dst = xT[:, dc, nb * 128:(nb + 1) * 128]
                        if dc % 2 == 0:
                            nc.vector.tensor_copy(out=dst, in_=tp)
                        else:
                            nc.scalar.copy(out=dst, in_=tp)

                # -- QKV^T [e, n]: for each 128-wide e-block (q | k | v)
                qT = qkv.tile([128, N], F32R, tag="qT")
                kT = qkv.tile([128, N], F32R, tag="kT")
                vT = qkv.tile([128, N], F32, tag="vT")
                for eb in range(3):
                    tgt = (qT, kT, vT)[eb]
                    for nc4 in range(4):
                        ps = psQ.tile([128, 512], F32, tag="q")
                        for dc in range(8):
                            nc.tensor.matmul(
                                ps,
                                w_r[:, dc, eb * 128:(eb + 1) * 128],
                                xT[:, dc, nc4 * 512:(nc4 + 1) * 512],
                                start=(dc == 0), stop=(dc == 7))
                        nc.vector.tensor_copy(
                            out=tgt[:, nc4 * 512:(nc4 + 1) * 512], in_=ps)

                # -- V' = [V_h | ones] per head, natural [k, d] layout
                Vp = qkv.tile([128, 16, 130], F32R, tag="Vp")
                for kb in range(16):
                    tp = psT.tile([128, 128], F32, tag="tp")
                    nc.tensor.transpose(tp, vT[:, kb * 128:(kb + 1) * 128],
                                        ident)
                    nc.vector.tensor_copy(out=Vp[:, kb, 0:64], in_=tp[:, 0:64])
                    nc.vector.tensor_copy(out=Vp[:, kb, 65:129],
                                          in_=tp[:, 64:128])
                    nc.vector.tensor_copy(out=Vp[:, kb, 64:65], in_=ones)
                    nc.vector.tensor_copy(out=Vp[:, kb, 129:130], in_=ones)

            # ============ attention phase (scoped PSUM pools) =============
            # banks: S 2x2 + OT 1x2 + denom 1x2 = 8
            with tc.tile_pool(name=f"psS{b}", bufs=2, space="PSUM") as psS, \
                 tc.tile_pool(name=f"psO{b}", bufs=2, space="PSUM") as psO:
                for qh in range(2):
                    OT = [psO.tile([65, 1024], F32, tag="OT", name=f"OT{_h}")
                          for _h in range(H_LOC)]
                    for kb in range(16):
                        S = [psS.tile([128, 1024], F32, tag="S", name=f"S{_h}")
                             for _h in range(H_LOC)]
                        for qc in range(2):
                            q0 = qh * 1024 + qc * 512
                            for h in range(H_LOC):
                                hp = 64 * h
                                nc.tensor.matmul(
                                    S[h][:, qc * 512:(qc + 1) * 512],
                                    kT[hp:hp + 64, kb * 128:(kb + 1) * 128],
                                    qT[hp:hp + 64, q0:q0 + 512],
                                    start=True, stop=True)
                        PT = [ptp.tile([128, 1024], F32R, tag="PT",
                                       name=f"PT{_h}") for _h in range(H_LOC)]
                        for h in range(H_LOC):
                            nc.scalar.activation(
                                out=PT[h], in_=S[h],
                                func=mybir.ActivationFunctionType.Exp,
                                scale=0.125)
                        for qc in range(2):
                            cs = slice(qc * 512, (qc + 1) * 512)
                            for h in range(H_LOC):
                                nc.tensor.matmul(
                                    OT[h][:, cs],
                                    Vp[:, kb, 65 * h:65 * h + 65],
                                    PT[h][:, cs],
                                    start=(kb == 0), stop=(kb == 15))
                    # epilogue: [q, 65] transpose + normalize + store
                    for h in range(H_LOC):
                        OT_sb = osb.tile([65, 1024], F32, tag="OT",
                                         name=f"OTs{h}")
                        if h == 0:
                            nc.vector.tensor_copy(out=OT_sb, in_=OT[h])
                        else:
                            nc.scalar.copy(out=OT_sb, in_=OT[h])
                        On = onp.tile([128, 8, HD], F32, tag="On",
                                      name=f"On{h}")
                        for t in range(8):
                            tp = psS.tile([128, 65], F32, tag="S", name="to")
                            nc.tensor.transpose(
                                tp, OT_sb[:, t * 128:(t + 1) * 128],
                                ident[0:65, 0:65])
                            rec = smp.tile([128, 1], F32, tag="rec")
                            nc.vector.reciprocal(rec, tp[:, 64:65])
                            nc.vector.tensor_scalar_mul(On[:, t, :],
                                                        tp[:, 0:64], rec)
                        nc.sync.dma_start(
                            out=out_ap[b, h, qh * 1024:(qh + 1) * 1024, :]
                            .rearrange("(t p) d -> p t d", p=128),
                            in_=On)


_CACHED_NC = None


def _build():
    global _CACHED_NC
    if _CACHED_NC is not None:
        return _CACHED_NC
    nc = bacc.Bacc("TRN2", target_bir_lowering=False, debug=False,
                   num_devices=NCORES)
    x = nc.dram_tensor("x", [B, N, DIM], F32, kind="ExternalInput").ap()
    w = nc.dram_tensor("w", [DIM, E], F32, kind="ExternalInput").ap()
    out = nc.dram_tensor("out", [B, H_LOC, N, HD], F32,
                         kind="ExternalOutput").ap()
    with tile.TileContext(nc) as tc:
        _emit(tc, out, x, w)
    nc.compile()
    _CACHED_NC = nc
    return nc


def _w_slice(w_qkv: np.ndarray, core: int) -> np.ndarray:
    cols = []
    for part in range(3):  # q, k, v column groups of w_qkv
        base = part * HEADS * HD + core * H_LOC * HD
        cols.append(w_qkv[:, base:base + H_LOC * HD])
    return np.ascontiguousarray(np.concatenate(cols, axis=1))


def kernel(x: np.ndarray, w_qkv: np.ndarray, _trace: bool = False):
    nc = _build()
    x = np.ascontiguousarray(x, dtype=np.float32)
    in_maps = [{"x": x, "w": _w_slice(w_qkv, i)} for i in range(NCORES)]
    res = run_bass_kernel_spmd(nc, in_maps, list(range(NCORES)), trace=_trace)
    out = np.empty((B, HEADS, N, HD), np.float32)
    for i in range(NCORES):
        out[:, i * H_LOC:(i + 1) * H_LOC] = res.results[i]["out"]
    if _trace:
        kernel.last_exec_time_ns = res.exec_time_ns
    return out


# revision 11
# speedup vs baseline: 1.3188x; 1.2019x over previous
"""Fused multi-head attention (QKV projection + softmax attention) on 8 TRN2
NeuronCores.

Problem: x [2, 2048, 1024] f32, w_qkv [1024, 3072] f32 ->
         out [2, 16, 2048, 64] f32   (16 heads, head_dim 64)

Sharding: tensor-parallel over heads. Each of the 8 cores owns 2 heads: it
gets the full x plus its [1024, 384] slice of w_qkv columns (q|k|v for its 2
heads) and computes its slice of the output independently. No collectives.

Per-core pipeline (all matmuls in float32r = full-rate fp32, ~1e-4 rel err):
  1. PE-transpose x[b] -> xT [d, n]  (PE matmul contracts over the partition
     dim, so both x@w operands must be d-major).
  2. QKV^T [e, n] = w.T @ x.T via PE, e-blocks of 128 (= 2 heads * 64).
  3. Attention with *transposed* scores, both heads interleaved so their
     K=64 score matmuls occupy disjoint PE row groups (concurrent) and the
     M=64 value matmuls occupy disjoint PE column groups (concurrent):
       S^T[k, q] = K^T.T @ Q^T     (no max-subtraction needed: scores are
       exp(S^T/8) -> P^T            ~N(0,1), far from fp32 overflow)
       O^T[d, q] (+= over k-blocks) = V.T @ P^T   both heads in one psum tile
       denom[q]  (+= over k-blocks) = ones.T @ P^T  [33,1024] psum, rows 0/32
     PE-transpose O^T and denom back to [q, *], multiply by 1/denom per row.
"""

import numpy as np

import concourse.bass as bass
import concourse.tile as tile
from concourse import bacc, mybir
from concourse.bass_utils import run_bass_kernel_spmd
from concourse.masks import make_identity

F32 = mybir.dt.float32
F32R = mybir.dt.float32r

B = 2
N = 2048
DIM = 1024
HEADS = 16
HD = 64
NCORES = 8
H_LOC = HEADS // NCORES  # 2 heads per core
E = 3 * H_LOC * HD       # 384 w columns per core
CACHEBUST = 5


def _emit(tc, out_ap, x_ap, w_ap):
    nc = tc.nc
    from contextlib import ExitStack
    ctx = ExitStack()
    with ctx:
        const = ctx.enter_context(tc.tile_pool(name="const", bufs=1))
        xtp = ctx.enter_context(tc.tile_pool(name="xtp", bufs=1))
        xst = ctx.enter_context(tc.tile_pool(name="xst", bufs=3))
        qkv = ctx.enter_context(tc.tile_pool(name="qkv", bufs=1))
        ptp = ctx.enter_context(tc.tile_pool(name="ptp", bufs=4))
        osb = ctx.enter_context(tc.tile_pool(name="osb", bufs=3))
        onp = ctx.enter_context(tc.tile_pool(name="onp", bufs=2))
        smp = ctx.enter_context(tc.tile_pool(name="smp", bufs=4))

        ident = const.tile([128, 128], F32)
        make_identity(nc, ident)
        ones = const.tile([128, 1], F32)
        nc.vector.memset(ones, 1.0)
        ones_r = const.tile([128, 1], F32R)
        nc.vector.tensor_copy(out=ones_r, in_=ones)
        _cb = const.tile([128, 1], F32, name="cb")
        nc.vector.memset(_cb, float(CACHEBUST))

        # w [1024, 384] -> [128, 8, 384], rounded to f32r for the PE
        w_sb = const.tile([128, 8, E], F32)
        nc.sync.dma_start(out=w_sb, in_=w_ap.rearrange("(c p) e -> p c e", p=128))
        w_r = const.tile([128, 8, E], F32R)
        nc.vector.tensor_copy(out=w_r, in_=w_sb)

        for b in range(B):
            # ============ projection phase (scoped PSUM pools) ============
            with tc.tile_pool(name=f"psT{b}", bufs=3, space="PSUM") as psT, \
                 tc.tile_pool(name=f"psQ{b}", bufs=2, space="PSUM") as psQ:
                # -- transpose x[b] -> xT chunks, QKV matmuls interleaved
                xT = xtp.tile([128, 8, N], F32R, tag="xT")
                qT = qkv.tile([128, N], F32R, tag="qT")
                kT = qkv.tile([128, N], F32R, tag="kT")
                vT = qkv.tile([128, N], F32, tag="vT")
                for nc4 in range(4):
                    for nb4 in range(4):
                        nb = nc4 * 4 + nb4
                        xs = xst.tile([128, DIM], F32, tag="xs")
                        nc.sync.dma_start(
                            out=xs, in_=x_ap[b, nb * 128:(nb + 1) * 128, :])
                        for dc in range(8):
                            tp = psT.tile([128, 128], F32, tag="tp")
                            nc.tensor.transpose(
                                tp, xs[:, dc * 128:(dc + 1) * 128], ident)
                            dst = xT[:, dc, nb * 128:(nb + 1) * 128]
                            if dc % 2 == 0:
                                nc.vector.tensor_copy(out=dst, in_=tp)
                            else:
                                nc.scalar.copy(out=dst, in_=tp)
                    for eb in range(3):
                        tgt = (qT, kT, vT)[eb]
                        ps = psQ.tile([128, 512], F32, tag="q")
                        for dc in range(8):
                            nc.tensor.matmul(
                                ps,
                                w_r[:, dc, eb * 128:(eb + 1) * 128],
                                xT[:, dc, nc4 * 512:(nc4 + 1) * 512],
                                start=(dc == 0), stop=(dc == 7))
                        nc.vector.tensor_copy(
                            out=tgt[:, nc4 * 512:(nc4 + 1) * 512], in_=ps)

                # -- V' = [V_h | ones] per head, natural [k, d] layout
                Vp = qkv.tile([128, 16, 130], F32R, tag="Vp")
                for kb in range(16):
                    tp = psT.tile([128, 128], F32, tag="tp")
                    nc.tensor.transpose(tp, vT[:, kb * 128:(kb + 1) * 128],
                                        ident)
                    nc.vector.tensor_copy(out=Vp[:, kb, 0:64], in_=tp[:, 0:64])
                    nc.vector.tensor_copy(out=Vp[:, kb, 65:129],
                                          in_=tp[:, 64:128])
                    nc.vector.tensor_copy(out=Vp[:, kb, 64:65], in_=ones)
                    nc.vector.tensor_copy(out=Vp[:, kb, 129:130], in_=ones)

            # ============ attention phase (scoped PSUM pools) =============
            # banks: S 2x2 + OT 1x2 + denom 1x2 = 8
            with tc.tile_pool(name=f"psS{b}", bufs=1, space="PSUM") as psS, \
                 tc.tile_pool(name=f"psO{b}", bufs=2, space="PSUM") as psO:
                for qh in range(2):
                    OT = [psO.tile([65, 1024], F32, tag="OT", name=f"OT{_h}")
                          for _h in range(H_LOC)]

                    def emit_pv(PT_p, kb_p):
                        for qc in range(2):
                            cs = slice(qc * 512, (qc + 1) * 512)
                            for h in range(H_LOC):
                                nc.tensor.matmul(
                                    OT[h][:, cs],
                                    Vp[:, kb_p, 65 * h:65 * h + 65],
                                    PT_p[h][:, cs],
                                    start=(kb_p == 0), stop=(kb_p == 15))

                    PT_prev = None
                    for kb in range(16):
                        S = psS.tile([128, 2048], F32, tag="S")
                        for qc in range(2):
                            q0 = qh * 1024 + qc * 512
                            for h in range(H_LOC):
                                hp = 64 * h
                                nc.tensor.matmul(
                                    S[:, h * 1024 + qc * 512:
                                      h * 1024 + (qc + 1) * 512],
                                    kT[hp:hp + 64, kb * 128:(kb + 1) * 128],
                                    qT[hp:hp + 64, q0:q0 + 512],
                                    start=True, stop=True)
                        if PT_prev is not None:
                            emit_pv(PT_prev, kb - 1)
                        PT = [ptp.tile([128, 1024], F32R, tag="PT",
                                       name=f"PT{_h}") for _h in range(H_LOC)]
                        for h in range(H_LOC):
                            nc.scalar.activation(
                                out=PT[h], in_=S[:, h * 1024:(h + 1) * 1024],
                                func=mybir.ActivationFunctionType.Exp,
                                scale=0.125)
                        PT_prev = PT
                    emit_pv(PT_prev, 15)
                    # epilogue: [q, 65] transpose + normalize + store
                    for h in range(H_LOC):
                        OT_sb = osb.tile([65, 1024], F32, tag="OT",
                                         name=f"OTs{h}")
                        if h == 0:
                            nc.vector.tensor_copy(out=OT_sb, in_=OT[h])
                        else:
                            nc.scalar.copy(out=OT_sb, in_=OT[h])
                        On = onp.tile([128, 8, HD], F32, tag="On",
                                      name=f"On{h}")
                        for t in range(8):
                            tp = psO.tile([128, 65], F32, tag="OT", name="to")
                            nc.tensor.transpose(
                                tp, OT_sb[:, t * 128:(t + 1) * 128],
                                ident[0:65, 0:65])
                            rec = smp.tile([128, 1], F32, tag="rec")
                            nc.vector.reciprocal(rec, tp[:, 64:65])
                            nc.vector.tensor_scalar_mul(On[:, t, :],
                                                        tp[:, 0:64], rec)
                        nc.sync.dma_start(
                            out=out_ap[b, h, qh * 1024:(qh + 1) * 1024, :]
                            .rearrange("(t p) d -> p t d", p=128),
                            in_=On)


_CACHED_NC = None


def _build():
    global _CACHED_NC
    if _CACHED_NC is not None:
        return _CACHED_NC
    nc = bacc.Bacc("TRN2", target_bir_lowering=False, debug=False,
                   num_devices=NCORES)
    x = nc.dram_tensor("x", [B, N, DIM], F32, kind="ExternalInput").ap()
    w = nc.dram_tensor("w", [DIM, E], F32, kind="ExternalInput").ap()
    out = nc.dram_tensor("out", [B, H_LOC, N, HD], F32,
                         kind="ExternalOutput").ap()
    with tile.TileContext(nc) as tc:
        _emit(tc, out, x, w)
    nc.compile()
    _CACHED_NC = nc
    return nc


def _w_slice(w_qkv: np.ndarray, core: int) -> np.ndarray:
    cols = []
    for part in range(3):  # q, k, v column groups of w_qkv
        base = part * HEADS * HD + core * H_LOC * HD
        cols.append(w_qkv[:, base:base + H_LOC * HD])
    return np.ascontiguousarray(np.concatenate(cols, axis=1))


def kernel(x: np.ndarray, w_qkv: np.ndarray, _trace: bool = False):
    nc = _build()
    x = np.ascontiguousarray(x, dtype=np.float32)
    in_maps = [{"x": x, "w": _w_slice(w_qkv, i)} for i in range(NCORES)]
    res = run_bass_kernel_spmd(nc, in_maps, list(range(NCORES)), trace=_trace)
    out = np.empty((B, HEADS, N, HD), np.float32)
    for i in range(NCORES):
        out[:, i * H_LOC:(i + 1) * H_LOC] = res.results[i]["out"]
    if _trace:
        kernel.last_exec_time_ns = res.exec_time_ns
    return out


# revision 13
# speedup vs baseline: 1.4081x; 1.0678x over previous
"""Fused multi-head attention (QKV projection + softmax attention) on 8 TRN2
NeuronCores.

Problem: x [2, 2048, 1024] f32, w_qkv [1024, 3072] f32 ->
         out [2, 16, 2048, 64] f32   (16 heads, head_dim 64)

Sharding: tensor-parallel over heads. Each of the 8 cores owns 2 heads: it
gets the full x plus its [1024, 384] slice of w_qkv columns (q|k|v for its 2
heads) and computes its slice of the output independently. No collectives.

Per-core pipeline (all matmuls in float32r = full-rate fp32, ~1e-4 rel err):
  1. PE-transpose x[b] -> xT [d, n]  (PE matmul contracts over the partition
     dim, so both x@w operands must be d-major).
  2. QKV^T [e, n] = w.T @ x.T via PE, e-blocks of 128 (= 2 heads * 64).
  3. Attention with *transposed* scores, both heads interleaved so their
     K=64 score matmuls occupy disjoint PE row groups (concurrent) and the
     M=64 value matmuls occupy disjoint PE column groups (concurrent):
       S^T[k, q] = K^T.T @ Q^T     (no max-subtraction needed: scores are
       exp(S^T/8) -> P^T            ~N(0,1), far from fp32 overflow)
       O^T[d, q] (+= over k-blocks) = V.T @ P^T   both heads in one psum tile
       denom[q]  (+= over k-blocks) = ones.T @ P^T  [33,1024] psum, rows 0/32
     PE-transpose O^T and denom back to [q, *], multiply by 1/denom per row.
"""

import numpy as np

import concourse.bass as bass
import concourse.tile as tile
from concourse import bacc, mybir
from concourse.bass_utils import run_bass_kernel_spmd
from concourse.masks import make_identity

F32 = mybir.dt.float32
F32R = mybir.dt.float32r

B = 2
N = 2048
DIM = 1024
HEADS = 16
HD = 64
NCORES = 8
H_LOC = HEADS // NCORES  # 2 heads per core
E = 3 * H_LOC * HD       # 384 w columns per core
CACHEBUST = 7


def _emit(tc, out_ap, x_ap, w_ap):
    nc = tc.nc
    from contextlib import ExitStack
    ctx = ExitStack()
    with ctx:
        const = ctx.enter_context(tc.tile_pool(name="const", bufs=1))
        xtp = ctx.enter_context(tc.tile_pool(name="xtp", bufs=1))
        xst = ctx.enter_context(tc.tile_pool(name="xst", bufs=3))
        qkv = ctx.enter_context(tc.tile_pool(name="qkv", bufs=1))
        ptp = ctx.enter_context(tc.tile_pool(name="ptp", bufs=3))
        osb = ctx.enter_context(tc.tile_pool(name="osb", bufs=3))
        onp = ctx.enter_context(tc.tile_pool(name="onp", bufs=2))
        smp = ctx.enter_context(tc.tile_pool(name="smp", bufs=4))

        ident = const.tile([128, 128], F32)
        make_identity(nc, ident)
        ones = const.tile([128, 1], F32)
        nc.vector.memset(ones, 1.0)
        ones_r = const.tile([128, 1], F32R)
        nc.vector.tensor_copy(out=ones_r, in_=ones)
        _cb = const.tile([128, 1], F32, name="cb")
        nc.vector.memset(_cb, float(CACHEBUST))

        # w [1024, 384] -> [128, 8, 384], rounded to f32r for the PE
        w_sb = const.tile([128, 8, E], F32)
        nc.sync.dma_start(out=w_sb, in_=w_ap.rearrange("(c p) e -> p c e", p=128))
        w_r = const.tile([128, 8, E], F32R)
        nc.vector.tensor_copy(out=w_r, in_=w_sb)

        for b in range(B):
            # ============ projection phase (scoped PSUM pools) ============
            with tc.tile_pool(name=f"psT{b}", bufs=5, space="PSUM") as psT, \
                 tc.tile_pool(name=f"psQ{b}", bufs=2, space="PSUM") as psQ:
                # -- transpose x[b] -> xT chunks, QKV matmuls interleaved
                xT = xtp.tile([128, 8, N], F32R, tag="xT")
                qT = qkv.tile([128, N], F32R, tag="qT")
                kT = qkv.tile([128, N], F32R, tag="kT")
                vT = qkv.tile([128, N], F32, tag="vT")
                for nc4 in range(4):
                    for nb4 in range(4):
                        nb = nc4 * 4 + nb4
                        xs = xst.tile([128, DIM], F32, tag="xs")
                        nc.sync.dma_start(
                            out=xs, in_=x_ap[b, nb * 128:(nb + 1) * 128, :])
                        for dc in range(8):
                            tp = psT.tile([128, 128], F32, tag="tp")
                            nc.tensor.transpose(
                                tp, xs[:, dc * 128:(dc + 1) * 128], ident)
                            dst = xT[:, dc, nb * 128:(nb + 1) * 128]
                            if dc % 2 == 0:
                                nc.vector.tensor_copy(out=dst, in_=tp)
                            else:
                                nc.scalar.copy(out=dst, in_=tp)
                    for eb in range(3):
                        tgt = (qT, kT, vT)[eb]
                        ps = psQ.tile([128, 512], F32, tag="q")
                        for dc in range(8):
                            nc.tensor.matmul(
                                ps,
                                w_r[:, dc, eb * 128:(eb + 1) * 128],
                                xT[:, dc, nc4 * 512:(nc4 + 1) * 512],
                                start=(dc == 0), stop=(dc == 7))
                        nc.vector.tensor_copy(
                            out=tgt[:, nc4 * 512:(nc4 + 1) * 512], in_=ps)

                # -- V' = [V_h | ones] per head, natural [k, d] layout
                Vp = qkv.tile([128, 16, 130], F32R, tag="Vp")
                for kb in range(16):
                    tp = psT.tile([128, 128], F32, tag="tp")
                    nc.tensor.transpose(tp, vT[:, kb * 128:(kb + 1) * 128],
                                        ident)
                    nc.vector.tensor_copy(out=Vp[:, kb, 0:64], in_=tp[:, 0:64])
                    nc.vector.tensor_copy(out=Vp[:, kb, 65:129],
                                          in_=tp[:, 64:128])
                    nc.vector.tensor_copy(out=Vp[:, kb, 64:65], in_=ones)
                    nc.vector.tensor_copy(out=Vp[:, kb, 129:130], in_=ones)

            # ============ attention phase (scoped PSUM pools) =============
            # banks: S 2x2 + OT 1x2 + denom 1x2 = 8
            with tc.tile_pool(name=f"psS{b}", bufs=1, space="PSUM") as psS, \
                 tc.tile_pool(name=f"psO{b}", bufs=2, space="PSUM") as psO:
                for qh in range(2):
                    OT = [psO.tile([65, 1024], F32, tag="OT", name=f"OT{_h}")
                          for _h in range(H_LOC)]

                    def emit_pv(PT_p, kb_p):
                        for qc in range(2):
                            for h in range(H_LOC):
                                pcs = slice(h * 1024 + qc * 512,
                                            h * 1024 + (qc + 1) * 512)
                                nc.tensor.matmul(
                                    OT[h][:, qc * 512:(qc + 1) * 512],
                                    Vp[:, kb_p, 65 * h:65 * h + 65],
                                    PT_p[:, pcs],
                                    start=(kb_p == 0), stop=(kb_p == 15))

                    PT_prev = None
                    for kb in range(16):
                        S = psS.tile([128, 2048], F32, tag="S")
                        for qc in range(2):
                            q0 = qh * 1024 + qc * 512
                            for h in range(H_LOC):
                                hp = 64 * h
                                nc.tensor.matmul(
                                    S[:, h * 1024 + qc * 512:
                                      h * 1024 + (qc + 1) * 512],
                                    kT[hp:hp + 64, kb * 128:(kb + 1) * 128],
                                    qT[hp:hp + 64, q0:q0 + 512],
                                    start=True, stop=True)
                        if PT_prev is not None:
                            emit_pv(PT_prev, kb - 1)
                        PT = ptp.tile([128, 2048], F32R, tag="PT")
                        nc.scalar.activation(
                            out=PT, in_=S,
                            func=mybir.ActivationFunctionType.Exp,
                            scale=0.125)
                        PT_prev = PT
                    emit_pv(PT_prev, 15)

                    # epilogue: [q, 65] transpose + normalize + store
                    for h in range(H_LOC):
                        OT_sb = osb.tile([65, 1024], F32, tag="OT",
                                         name=f"OTs{h}")
                        if h == 0:
                            nc.vector.tensor_copy(out=OT_sb, in_=OT[h])
                        else:
                            nc.scalar.copy(out=OT_sb, in_=OT[h])
                        On = onp.tile([128, 8, HD], F32, tag="On",
                                      name=f"On{h}")
                        for t in range(8):
                            tp = psO.tile([128, 65], F32, tag="OT", name="to")
                            nc.tensor.transpose(
                                tp, OT_sb[:, t * 128:(t + 1) * 128],
                                ident[0:65, 0:65])
                            rec = smp.tile([128, 1], F32, tag="rec")
                            nc.vector.reciprocal(rec, tp[:, 64:65])
                            nc.vector.tensor_scalar_mul(On[:, t, :],
                                                        tp[:, 0:64], rec)
                        nc.sync.dma_start(
                            out=out_ap[b, h, qh * 1024:(qh + 1) * 1024, :]
                            .rearrange("(t p) d -> p t d", p=128),
                            in_=On)


_CACHED_NC = None


def _build():
    global _CACHED_NC
    if _CACHED_NC is not None:
        return _CACHED_NC
    nc = bacc.Bacc("TRN2", target_bir_lowering=False, debug=False,
                   num_devices=NCORES)
    x = nc.dram_tensor("x", [B, N, DIM], F32, kind="ExternalInput").ap()
    w = nc.dram_tensor("w", [DIM, E], F32, kind="ExternalInput").ap()
    out = nc.dram_tensor("out", [B, H_LOC, N, HD], F32,
                         kind="ExternalOutput").ap()
    with tile.TileContext(nc) as tc:
        _emit(tc, out, x, w)
    nc.compile()
    _CACHED_NC = nc
    return nc


def _w_slice(w_qkv: np.ndarray, core: int) -> np.ndarray:
    cols = []
    for part in range(3):  # q, k, v column groups of w_qkv
        base = part * HEADS * HD + core * H_LOC * HD
        cols.append(w_qkv[:, base:base + H_LOC * HD])
    return np.ascontiguousarray(np.concatenate(cols, axis=1))


def kernel(x: np.ndarray, w_qkv: np.ndarray, _trace: bool = False):
    nc = _build()
    x = np.ascontiguousarray(x, dtype=np.float32)
    in_maps = [{"x": x, "w": _w_slice(w_qkv, i)} for i in range(NCORES)]
    res = run_bass_kernel_spmd(nc, in_maps, list(range(NCORES)), trace=_trace)
    out = np.empty((B, HEADS, N, HD), np.float32)
    for i in range(NCORES):
        out[:, i * H_LOC:(i + 1) * H_LOC] = res.results[i]["out"]
    if _trace:
        kernel.last_exec_time_ns = res.exec_time_ns
    return out


# revision 14
# speedup vs baseline: 1.6169x; 1.1483x over previous
"""Fused multi-head attention (QKV projection + softmax attention) on 8 TRN2
NeuronCores.

Problem: x [2, 2048, 1024] f32, w_qkv [1024, 3072] f32 ->
         out [2, 16, 2048, 64] f32   (16 heads, head_dim 64)

Sharding: tensor-parallel over heads. Each of the 8 cores owns 2 heads: it
gets the full x plus its [1024, 384] slice of w_qkv columns (q|k|v for its 2
heads) and computes its slice of the output independently. No collectives.

Per-core pipeline (all matmuls in float32r = full-rate fp32, ~1e-4 rel err):
  1. PE-transpose x[b] -> xT [d, n]  (PE matmul contracts over the partition
     dim, so both x@w operands must be d-major).
  2. QKV^T [e, n] = w.T @ x.T via PE, e-blocks of 128 (= 2 heads * 64).
  3. Attention with *transposed* scores, both heads interleaved so their
     K=64 score matmuls occupy disjoint PE row groups (concurrent) and the
     M=64 value matmuls occupy disjoint PE column groups (concurrent):
       S^T[k, q] = K^T.T @ Q^T     (no max-subtraction needed: scores are
       exp(S^T/8) -> P^T            ~N(0,1), far from fp32 overflow)
       O^T[d, q] (+= over k-blocks) = V.T @ P^T   both heads in one psum tile
       denom[q]  (+= over k-blocks) = ones.T @ P^T  [33,1024] psum, rows 0/32
     PE-transpose O^T and denom back to [q, *], multiply by 1/denom per row.
"""

import numpy as np

import concourse.bass as bass
import concourse.tile as tile
from concourse import bacc, mybir
from concourse.bass_utils import run_bass_kernel_spmd
from concourse.masks import make_identity

F32 = mybir.dt.float32
F32R = mybir.dt.float32r

B = 2
N = 2048
DIM = 1024
HEADS = 16
HD = 64
NCORES = 8
H_LOC = HEADS // NCORES  # 2 heads per core
E = 3 * H_LOC * HD       # 384 w columns per core
CACHEBUST = 8


def _emit(tc, out_ap, x_ap, w_ap):
    nc = tc.nc
    from contextlib import ExitStack
    ctx = ExitStack()
    with ctx:
        const = ctx.enter_context(tc.tile_pool(name="const", bufs=1))
        xtp = ctx.enter_context(tc.tile_pool(name="xtp", bufs=1))
        xst = ctx.enter_context(tc.tile_pool(name="xst", bufs=3))
        qkv = ctx.enter_context(tc.tile_pool(name="qkv", bufs=1))
        ptp = ctx.enter_context(tc.tile_pool(name="ptp", bufs=4))
        osb = ctx.enter_context(tc.tile_pool(name="osb", bufs=8))
        onp = ctx.enter_context(tc.tile_pool(name="onp", bufs=2))
        smp = ctx.enter_context(tc.tile_pool(name="smp", bufs=4))

        ident = const.tile([128, 128], F32)
        make_identity(nc, ident)
        ones = const.tile([128, 1], F32)
        nc.vector.memset(ones, 1.0)
        ones_r = const.tile([128, 1], F32R)
        nc.vector.tensor_copy(out=ones_r, in_=ones)
        _cb = const.tile([128, 1], F32, name="cb")
        nc.vector.memset(_cb, float(CACHEBUST))

        # w [1024, 384] -> [128, 8, 384], rounded to f32r for the PE
        w_sb = const.tile([128, 8, E], F32)
        nc.sync.dma_start(out=w_sb, in_=w_ap.rearrange("(c p) e -> p c e", p=128))
        w_r = const.tile([128, 8, E], F32R)
        nc.vector.tensor_copy(out=w_r, in_=w_sb)

        for b in range(B):
            # ============ projection phase (scoped PSUM pools) ============
            with tc.tile_pool(name=f"psT{b}", bufs=5, space="PSUM") as psT, \
                 tc.tile_pool(name=f"psQ{b}", bufs=2, space="PSUM") as psQ:
                # -- transpose x[b] -> xT chunks, QKV matmuls interleaved
                xT = xtp.tile([128, 8, N], F32R, tag="xT")
                qT = qkv.tile([128, N], F32R, tag="qT")
                kT = qkv.tile([128, N], F32R, tag="kT")
                vT = qkv.tile([128, N], F32, tag="vT")
                for nc4 in range(4):
                    for nb4 in range(4):
                        nb = nc4 * 4 + nb4
                        xs = xst.tile([128, DIM], F32, tag="xs")
                        nc.sync.dma_start(
                            out=xs, in_=x_ap[b, nb * 128:(nb + 1) * 128, :])
                        for dc in range(8):
                            tp = psT.tile([128, 128], F32, tag="tp")
                            nc.tensor.transpose(
                                tp, xs[:, dc * 128:(dc + 1) * 128], ident)
                            dst = xT[:, dc, nb * 128:(nb + 1) * 128]
                            if dc % 2 == 0:
                                nc.vector.tensor_copy(out=dst, in_=tp)
                            else:
                                nc.scalar.copy(out=dst, in_=tp)
                    for eb in range(3):
                        tgt = (qT, kT, vT)[eb]
                        ps = psQ.tile([128, 512], F32, tag="q")
                        for dc in range(8):
                            nc.tensor.matmul(
                                ps,
                                w_r[:, dc, eb * 128:(eb + 1) * 128],
                                xT[:, dc, nc4 * 512:(nc4 + 1) * 512],
                                start=(dc == 0), stop=(dc == 7))
                        nc.vector.tensor_copy(
                            out=tgt[:, nc4 * 512:(nc4 + 1) * 512], in_=ps)

                # -- V' = [V_h | ones] per head, natural [k, d] layout
                Vp = qkv.tile([128, 16, 130], F32R, tag="Vp")
                for kb in range(16):
                    tp = psT.tile([128, 128], F32, tag="tp")
                    nc.tensor.transpose(tp, vT[:, kb * 128:(kb + 1) * 128],
                                        ident)
                    nc.vector.tensor_copy(out=Vp[:, kb, 0:64], in_=tp[:, 0:64])
                    nc.vector.tensor_copy(out=Vp[:, kb, 65:129],
                                          in_=tp[:, 64:128])
                    nc.vector.tensor_copy(out=Vp[:, kb, 64:65], in_=ones)
                    nc.vector.tensor_copy(out=Vp[:, kb, 129:130], in_=ones)

            # ============ attention phase (scoped PSUM pools) =============
            # banks: S 2x2 + OT 1x2 + denom 1x2 = 8
            with tc.tile_pool(name=f"psS{b}", bufs=2, space="PSUM") as psS, \
                 tc.tile_pool(name=f"psO{b}", bufs=2, space="PSUM") as psO:
                epilogs = []
                for qh in range(2):
                    OT = [psO.tile([65, 1024], F32, tag="OT", name=f"OT{_h}")
                          for _h in range(H_LOC)]

                    def emit_pv(PT_p, kb_p, qc):
                        for h in range(H_LOC):
                            nc.tensor.matmul(
                                OT[h][:, qc * 512:(qc + 1) * 512],
                                Vp[:, kb_p, 65 * h:65 * h + 65],
                                PT_p[:, h * 512:(h + 1) * 512],
                                start=(kb_p == 0), stop=(kb_p == 15))

                    PT_prev = None
                    for kb in range(16):
                        Sab = []
                        PTab = []
                        for qc in range(2):
                            S = psS.tile([128, 1024], F32, tag="S",
                                         name=f"S{qc}")
                            q0 = qh * 1024 + qc * 512
                            for h in range(H_LOC):
                                hp = 64 * h
                                nc.tensor.matmul(
                                    S[:, h * 512:(h + 1) * 512],
                                    kT[hp:hp + 64, kb * 128:(kb + 1) * 128],
                                    qT[hp:hp + 64, q0:q0 + 512],
                                    start=True, stop=True)
                            if PT_prev is not None:
                                emit_pv(PT_prev[qc], kb - 1, qc)
                            Sab.append(S)
                        for qc in range(2):
                            PT = ptp.tile([128, 1024], F32R, tag="PT",
                                          name=f"PT{qc}")
                            nc.scalar.activation(
                                out=PT, in_=Sab[qc],
                                func=mybir.ActivationFunctionType.Exp,
                                scale=0.125)
                            PTab.append(PT)
                        PT_prev = PTab
                    for qc in range(2):
                        emit_pv(PT_prev[qc], 15, qc)

                    # copy psum out now; defer transpose+normalize to batch end
                    qh_sb = []
                    for h in range(H_LOC):
                        OT_sb = osb.tile([65, 1024], F32, tag="OT",
                                         name=f"OTs{qh}{h}")
                        if h == 0:
                            nc.vector.tensor_copy(out=OT_sb, in_=OT[h])
                        else:
                            nc.scalar.copy(out=OT_sb, in_=OT[h])
                        qh_sb.append(OT_sb)
                    epilogs.append((qh, qh_sb))

                # deferred epilogues: transpose to [q, 65], normalize, store
                for qh, qh_sb in epilogs:
                    for h in range(H_LOC):
                        OT_sb = qh_sb[h]
                        On = onp.tile([128, 8, HD], F32, tag="On",
                                      name=f"On{qh}{h}")
                        for t in range(8):
                            tp = psO.tile([128, 65], F32, tag="OT", name="to")
                            nc.tensor.transpose(
                                tp, OT_sb[:, t * 128:(t + 1) * 128],
                                ident[0:65, 0:65])
                            rec = smp.tile([128, 1], F32, tag="rec")
                            nc.vector.reciprocal(rec, tp[:, 64:65])
                            nc.vector.tensor_scalar_mul(On[:, t, :],
                                                        tp[:, 0:64], rec)
                        nc.sync.dma_start(
                            out=out_ap[b, h, qh * 1024:(qh + 1) * 1024, :]
                            .rearrange("(t p) d -> p t d", p=128),
                            in_=On)


_CACHED_NC = None


def _build():
    global _CACHED_NC
    if _CACHED_NC is not None:
        return _CACHED_NC
    nc = bacc.Bacc("TRN2", target_bir_lowering=False, debug=False,
                   num_devices=NCORES)
    x = nc.dram_tensor("x", [B, N, DIM], F32, kind="ExternalInput").ap()
    w = nc.dram_tensor("w", [DIM, E], F32, kind="ExternalInput").ap()
    out = nc.dram_tensor("out", [B, H_LOC, N, HD], F32,
                         kind="ExternalOutput").ap()
    with tile.TileContext(nc) as tc:
        _emit(tc, out, x, w)
    nc.compile()
    _CACHED_NC = nc
    return nc


def _w_slice(w_qkv: np.ndarray, core: int) -> np.ndarray:
    cols = []
    for part in range(3):  # q, k, v column groups of w_qkv
        base = part * HEADS * HD + core * H_LOC * HD
        cols.append(w_qkv[:, base:base + H_LOC * HD])
    return np.ascontiguousarray(np.concatenate(cols, axis=1))


def kernel(x: np.ndarray, w_qkv: np.ndarray, _trace: bool = False):
    nc = _build()
    x = np.ascontiguousarray(x, dtype=np.float32)
    in_maps = [{"x": x, "w": _w_slice(w_qkv, i)} for i in range(NCORES)]
    res = run_bass_kernel_spmd(nc, in_maps, list(range(NCORES)), trace=_trace)
    out = np.empty((B, HEADS, N, HD), np.float32)
    for i in range(NCORES):
        out[:, i * H_LOC:(i + 1) * H_LOC] = res.results[i]["out"]
    if _trace:
        kernel.last_exec_time_ns = res.exec_time_ns
    return out
